# revision 2
# baseline (speedup 1.0000x reference)
"""Trainium2 Bass kernel v3 for nn_EnergyEwald.

vs v2: drops the k-space recursion (small per-half DVE ops measured 3-4x
the model; net DVE regression). All 510 half-set kvec columns get direct
Sin/Cos. Keeps: fp16 cs + accumulate matmuls (one PE pass instead of
fp32's two), bank-aligned tt blocks, ACT Abs_reciprocal_sqrt for 1/d,
GPSIMD d^2 concurrent with the K phase, table-set batching (R-phase ACT
emitted after the K loop), DVE staging copies (no ACT Copy thrash).
New: alternate |g| between ACT Abs and DVE (subtract + int-view bitand)
to balance the two engines.
"""

import math
import os
import sys
from contextlib import ExitStack

import numpy as np

for _p in ("/opt/trn_rl_repo", "/root/.axon_site/_ro/trn_rl_repo"):
    if os.path.isdir(_p) and _p not in sys.path:
        sys.path.insert(0, _p)

import concourse.tile as tile  # noqa: E402
from concourse import bacc, bass_utils, mybir  # noqa: E402

KE = 14.3996
ALPHA = 0.3
CUTOFF = 10.0
SQA = math.sqrt(ALPHA)
FCUT = math.erfc(SQA * CUTOFF) / CUTOFF
TWO_PI = 2.0 * math.pi
MAGIC = 12582912.0  # 1.5 * 2**23: float32 round-to-nearest-integer trick

N_CORES = 8
N_ATOMS = 100000
N_PAIRS = 6400000
N_MOL = 64

APC = N_ATOMS // N_CORES          # atoms per core = 12500
ACH = 98                          # 128-atom chunks per core
APAD = ACH * 128                  # 12544
NSUP = ACH // 2                   # super-chunks of 256 atoms = 49

NRT = 7                           # real-space tiles
PAD_X = 50.0                      # pad pair distance -> masked out

F32 = mybir.dt.float32
F32R = mybir.dt.float32r
F16 = mybir.dt.float16
U32 = mybir.dt.uint32

_PROG_CACHE = {}

# fraction of superchunks whose |g| runs on DVE instead of ACT: s % ADEN < ANUM
ANUM, ADEN = 1, 2


def _build_program(sig, reps=1):
    """sig = (NK, CCOLS): kvec cols and real-space pair columns."""
    NK, CCOLS = sig
    CT = CCOLS // NRT
    CH = CT // 2
    AluOp = mybir.AluOpType
    AF = mybir.ActivationFunctionType

    nc = bacc.Bacc("TRN2", target_bir_lowering=False, debug=False,
                   num_devices=N_CORES)

    def din(name, shape, dt=F32):
        return nc.dram_tensor(name, shape, dt, kind="ExternalInput").ap()

    def dout(name, shape, dt=F32):
        return nc.dram_tensor(name, shape, dt, kind="ExternalOutput").ap()

    u_t = din("u_t", [3, NSUP * 256], F32R)  # frac coords, [3, 12544]
    kv_t = din("kv_t", [3, NK], F32R)      # kvec rows
    qoh = din("qoh", [128, ACH * 64], F16)  # q-scaled one-hot, packed
    xs = din("xs", [128, CCOLS])
    ys = din("ys", [128, CCOLS])
    zs = din("zs", [128, CCOLS])
    qq = din("qq", [128, CCOLS])           # q[i]*q[j] per pair

    o_qr = dout("o_qr", [64, NK])          # sum q*cos per (mol, kvec col)
    o_qi = dout("o_qi", [64, NK])          # sum q*sin
    o_cs = dout("o_cs", [1, CCOLS])        # per-column pair-potential sums

    with tile.TileContext(nc, trace_sim=False) as tc, ExitStack() as ctx:
        pers = ctx.enter_context(tc.tile_pool(name="pers", bufs=1))
        io = ctx.enter_context(tc.tile_pool(name="io", bufs=2))
        kio = ctx.enter_context(tc.tile_pool(name="kio", bufs=2))
        tmp = ctx.enter_context(tc.tile_pool(name="tmp", bufs=2))
        rsp = ctx.enter_context(tc.tile_pool(name="rsp", bufs=2))
        ps_t = ctx.enter_context(tc.tile_pool(name="ps_t", bufs=2,
                                              space="PSUM"))
        ps_acc = ctx.enter_context(tc.tile_pool(name="ps_acc", bufs=1,
                                                space="PSUM"))
        ps_cs = ctx.enter_context(tc.tile_pool(name="ps_cs", bufs=2,
                                               space="PSUM"))

        # persistent SBUF
        kv_sb = pers.tile([3, NK], F32R)
        qoh_sb = pers.tile([128, ACH * 64], F16)
        d2buf = pers.tile([128, CCOLS], F32)   # d^2
        dbuf = pers.tile([128, CCOLS], F32)    # d, then 1/d
        ebuf = pers.tile([128, CCOLS], F32)    # -erf(sqa*d)
        ones32 = pers.tile([128, 1], F32)
        onesr = pers.tile([128, 1], F32R)
        halfpi = pers.tile([128, 1], F32)
        negq = pers.tile([128, 1], F32)
        negfc = pers.tile([128, 1], F32)
        nc.vector.memset(negfc[:], -FCUT)

        nc.vector.memset(ones32[:], 1.0)
        nc.vector.tensor_copy(onesr[:], ones32[:])
        nc.vector.memset(halfpi[:], math.pi / 2)
        nc.vector.memset(negq[:], -0.25)

        # one-time setup DMAs (outside the rep body)
        nc.sync.dma_start(kv_sb[:], kv_t[:])
        nc.sync.dma_start(qoh_sb[:], qoh[:])

        def _emit_once():
            qr_ps = ps_acc.tile([64, NK], F32, tag="qr")
            qi_ps = ps_acc.tile([64, NK], F32, tag="qi")

            def _emit_d2_tile(i):
                # stream xyz, compute d^2 on GPSIMD into d2buf
                sl = slice(i * CT, (i + 1) * CT)
                xt = io.tile([128, CT], F32, tag="xt")
                nc.sync.dma_start(xt[:], xs[:, sl])
                yt = io.tile([128, CT], F32, tag="yt")
                nc.sync.dma_start(yt[:], ys[:, sl])
                zt = io.tile([128, CT], F32, tag="zt")
                nc.sync.dma_start(zt[:], zs[:, sl])
                sq = rsp.tile([128, CT], F32, tag="sq")
                t1 = rsp.tile([128, CT], F32, tag="t1")
                nc.gpsimd.tensor_tensor(sq[:], xt[:], xt[:], AluOp.mult)
                nc.gpsimd.tensor_tensor(t1[:], yt[:], yt[:], AluOp.mult)
                nc.gpsimd.tensor_tensor(sq[:], sq[:], t1[:], AluOp.add)
                nc.gpsimd.tensor_tensor(t1[:], zt[:], zt[:], AluOp.mult)
                nc.gpsimd.tensor_tensor(d2buf[:, sl], sq[:], t1[:],
                                        AluOp.add)

            def _emit_rc_tail(i):
                # f = (1 - erf)(1/d); pot = relu(f - FCUT) * qq; col-reduce
                sl = slice(i * CT, (i + 1) * CT)
                f = rsp.tile([128, CT], F32, tag="sq")
                nc.vector.scalar_tensor_tensor(f[:], ebuf[:, sl], 1.0,
                                               dbuf[:, sl],
                                               AluOp.add, AluOp.mult)
                p = rsp.tile([128, CT], F32, tag="t1")
                nc.vector.tensor_scalar(p[:], f[:], FCUT, 0.0,
                                        AluOp.subtract, AluOp.max)
                qqt = io.tile([128, CT], F32, tag="xt")
                nc.sync.dma_start(qqt[:], qq[:, sl])
                pot = rsp.tile([128, CT], F32R, tag="sq")
                nc.gpsimd.tensor_tensor(pot[:], p[:], qqt[:], AluOp.mult)
                for j in range(2):
                    cps = ps_cs.tile([1, CH], F32, tag="cs")
                    nc.tensor.matmul(cps[:], onesr[:],
                                     pot[:, j * CH:(j + 1) * CH],
                                     start=True, stop=True)
                    cs_sb = rsp.tile([1, CH], F32, tag="cso")
                    nc.scalar.activation(cs_sb[:], cps[:], AF.Copy)
                    lo = i * CT + j * CH
                    nc.sync.dma_start(o_cs[0:1, lo:lo + CH], cs_sb[:])

            u_g = [None] * 7

            def _emit_ksup(s):
                # ---- phases: tt[h] = u_chunk . kv (blocks bank-aligned) ----
                g, c0 = s // 7, (s % 7) * 256
                if s % 7 == 0:
                    ugt = kio.tile([3, 7 * 256], F32R, tag="ug")
                    u_g[g] = ugt
                    nc.sync.dma_start(ugt[:],
                                      u_t[:, g * 1792:(g + 1) * 1792])
                tt = ps_t.tile([128, 1024], F32, tag="tt")
                for h in range(2):
                    lhs = u_g[g][:, c0 + h * 128:c0 + (h + 1) * 128]
                    # f32r: single PE pass (~19-bit mantissa, ample for
                    # phases); plain f32 lowers to two hi/lo passes
                    nc.tensor.matmul(tt[:, h * 512:h * 512 + NK], lhs,
                                     kv_sb[:], start=True, stop=True)
                tt3 = tt[:].rearrange("p (a b) -> p a b", a=2)[:, :, 0:NK]
                # ---- range reduce: g = t - round(t); w = 0.25 - g ----
                rnd = tmp.tile([128, 2 * NK], F32, tag="rnd")
                rnd3 = rnd[:].rearrange("p (a b) -> p a b", a=2)
                nc.vector.tensor_scalar(rnd3, tt3, MAGIC, MAGIC,
                                        AluOp.add, AluOp.subtract)
                wga = tmp.tile([128, 4 * NK], F32, tag="wga")
                # g = t - round(t) in [-0.5, 0.5]
                nc.vector.scalar_tensor_tensor(
                    wga[:, 0:2 * NK].rearrange("p (a b) -> p a b", a=2),
                    tt3, 0.0, rnd3, AluOp.add, AluOp.subtract)
                # |g|: alternate DVE bitand / ACT Abs to balance engines
                if s % 2 == 0:
                    nc.vector.tensor_scalar(
                        wga[:, 2 * NK:4 * NK].bitcast(U32),
                        wga[:, 0:2 * NK].bitcast(U32),
                        0x7FFFFFFF, None, AluOp.bitwise_and)
                else:
                    nc.scalar.activation(wga[:, 2 * NK:4 * NK],
                                         wga[:, 0:2 * NK], AF.Abs)
                # ---- two Sins: sin(2pi g) on g; cos = sin(pi/2 - 2pi|g|) ----
                cs_t = kio.tile([128, 4 * NK], F16, tag="cs")
                nc.scalar.activation(cs_t[:, 0:2 * NK], wga[:, 0:2 * NK],
                                     AF.Sin, scale=TWO_PI)
                nc.scalar.activation(cs_t[:, 2 * NK:4 * NK],
                                     wga[:, 2 * NK:4 * NK],
                                     AF.Sin, scale=-TWO_PI, bias=halfpi[:])
                # ---- accumulate segment sums ----
                for h in range(2):
                    ch = 2 * s + h
                    lhs = qoh_sb[:, ch * 64:(ch + 1) * 64]
                    first = (ch == 0)
                    last = (ch == ACH - 1)
                    nc.tensor.matmul(qr_ps[:], lhs,
                                     cs_t[:, (2 + h) * NK:(3 + h) * NK],
                                     start=first, stop=last,
                                     skip_group_check=True)
                    nc.tensor.matmul(qi_ps[:], lhs,
                                     cs_t[:, h * NK:(h + 1) * NK],
                                     start=first, stop=last,
                                     skip_group_check=True)

            # ---------------- schedule ----------------
            for s in range(NSUP):
                if s % 5 == 1 and s // 5 < NRT:
                    _emit_d2_tile(s // 5)
                _emit_ksup(s)

            # Gate the R-phase ACT work behind the last superchunk's Sin so
            # the scheduler can't interleave it into ACT gaps mid-K (each
            # slot-in costs two ~2.7us table-set loads). The token op writes
            # one column of each tile's d2buf slice, creating a RAW dep.
            for i in range(NRT):
                nc.vector.tensor_scalar(d2buf[:, i * CT:i * CT + 1],
                                        d2buf[:, i * CT:i * CT + 1],
                                        0.0, None, AluOp.add)
            # R phases (batched by ACT table set, after the K loop)
            for i in range(NRT):
                sl = slice(i * CT, (i + 1) * CT)
                nc.scalar.activation(dbuf[:, sl], d2buf[:, sl], AF.Sqrt)
            for i in range(NRT):
                sl = slice(i * CT, (i + 1) * CT)
                nc.scalar.activation(ebuf[:, sl], dbuf[:, sl], AF.Erf,
                                     scale=-SQA)
            for i in range(NRT):
                sl = slice(i * CT, (i + 1) * CT)
                nc.scalar.activation(dbuf[:, sl], d2buf[:, sl],
                                     AF.Abs_reciprocal_sqrt)
            for i in range(NRT):
                _emit_rc_tail(i)

            # drain accumulators via SBUF staging
            qr_sb = pers.tile([64, NK], F32, tag="qro")
            qi_sb = pers.tile([64, NK], F32, tag="qio")
            nc.scalar.activation(qr_sb[:], qr_ps[:], AF.Copy)
            nc.scalar.activation(qi_sb[:], qi_ps[:], AF.Copy)
            nc.sync.dma_start(o_qr[:], qr_sb[:])
            nc.sync.dma_start(o_qi[:], qi_sb[:])

        for _rep in range(reps):
            _emit_once()

    nc.compile()
    return nc


def _get_program(sig, reps=1):
    key = (sig, reps)
    if key not in _PROG_CACHE:
        _PROG_CACHE[key] = _build_program(sig, reps)
    return _PROG_CACHE[key]


def _half_kvecs(kvecs):
    """Pick one of each +-k pair (lexicographically positive)."""
    nk = kvecs.shape[0]
    key = {tuple(v): i for i, v in enumerate(kvecs)}
    sel = []
    for i, v in enumerate(kvecs):
        t = tuple(v)
        tn = tuple(-x for x in v)
        if tn not in key:
            return None
        if t > (0.0, 0.0, 0.0):
            sel.append(i)
    if len(sel) * 2 != nk:
        return None
    return np.asarray(sel)


def prepare(inputs):
    """Host prep: returns (nc, in_maps, combine_fn)."""
    q = np.asarray(inputs["partial_charges"], np.float32)[:, 0]
    Rij = np.asarray(inputs["Rij"], np.float32)
    R = np.asarray(inputs["R"], np.float32)
    cell = np.asarray(inputs["cell"], np.float32)
    kvecs = np.asarray(inputs["kvecs"], np.float32)
    idx_m = np.asarray(inputs["idx_m"]).astype(np.int64)
    idx_i = np.asarray(inputs["idx_i"]).astype(np.int64)
    idx_j = np.asarray(inputs["idx_j"]).astype(np.int64)

    sel = _half_kvecs(kvecs)
    assert sel is not None, "kvec set is not +-symmetric"
    kv_use = kvecs[sel]
    wk = 2.0
    NK = kv_use.shape[0]
    assert NK <= 512, "kvec half-set exceeds one PSUM bank"

    # ---------- host prep: reciprocal space ----------
    invc = np.linalg.inv(cell.astype(np.float64))
    u_all = np.einsum("ae,aed->ad", R, invc[idx_m]).astype(np.float32)
    kv_t_np = np.ascontiguousarray(kv_use.T.astype(np.float32))  # [3, NK]

    # ---------- host prep: real space ----------
    mol_pair = idx_m[idx_i].astype(np.int32)
    qq_pair = q[idx_i] * q[idx_j]
    order = np.argsort(mol_pair, kind="stable")
    xs_s = Rij[order, 0]
    ys_s = Rij[order, 1]
    zs_s = Rij[order, 2]
    qq_s = qq_pair[order]
    counts = np.bincount(mol_pair, minlength=N_MOL)
    starts = np.concatenate(([0], np.cumsum(counts)))

    # real-space column budget: worst core, rounded up to tiles of even CT
    worst = 0
    for c in range(N_CORES):
        tot = 0
        for m in range(N_MOL):
            n = counts[m]
            share = (n + N_CORES - 1) // N_CORES
            ncm = max(min(n, (c + 1) * share) - c * share, 0)
            tot += (ncm + 127) // 128
        worst = max(worst, tot)
    CT = -(-worst // (NRT * 4)) * 4
    CCOLS = NRT * CT
    nc = _get_program((NK, CCOLS))

    in_maps = []
    colmols = []
    SLOTS = CCOLS * 128
    for c in range(N_CORES):
        gidx = np.full(SLOTS, -1, np.int64)   # [p, col] flattened p*CCOLS+col
        colmol = np.full(CCOLS, -1, np.int32)
        col0 = 0
        for m in range(N_MOL):
            n = counts[m]
            share = (n + N_CORES - 1) // N_CORES
            lo = starts[m] + c * share
            hi = min(starts[m] + n, lo + share)
            ncm = max(hi - lo, 0)
            if ncm == 0:
                continue
            ncols = (ncm + 127) // 128
            js = np.arange(ncm)
            gidx[(js % 128) * CCOLS + col0 + js // 128] = lo + js
            colmol[col0:col0 + ncols] = m
            col0 += ncols
        assert col0 <= CCOLS, f"column overflow: {col0} > {CCOLS}"
        valid = gidx >= 0
        gv = gidx[valid]

        def fill(src, pad, dt=np.float32):
            a = np.full(SLOTS, pad, dt)
            a[valid] = src[gv].astype(dt)
            return a.reshape(128, CCOLS)

        # atoms for this core: round-robin slice keeps mol-sorted order
        a_ids = np.arange(c, N_ATOMS, N_CORES)
        u_core = np.zeros((APAD, 3), np.float32)
        u_core[:APC] = u_all[a_ids]
        q_core = np.zeros(APAD, np.float32)
        q_core[:APC] = q[a_ids]
        m_core = np.zeros(APAD, np.int64)
        m_core[:APC] = idx_m[a_ids]
        qoh_np = np.zeros((APAD, 64), np.float16)
        qoh_np[np.arange(APAD), m_core] = q_core.astype(np.float16)
        qoh_pack = np.ascontiguousarray(
            qoh_np.reshape(ACH, 128, 64).transpose(1, 0, 2)
            .reshape(128, ACH * 64))
        u_pack = np.ascontiguousarray(u_core.T)

        in_maps.append({
            "u_t": u_pack,
            "kv_t": kv_t_np,
            "qoh": qoh_pack,
            "xs": fill(xs_s, PAD_X),
            "ys": fill(ys_s, 0.0),
            "zs": fill(zs_s, 0.0),
            "qq": fill(qq_s, 0.0),
        })
        colmols.append(colmol)

    self_q2_host = np.bincount(idx_m, weights=(q.astype(np.float64) ** 2),
                               minlength=N_MOL)

    def combine(results):
        q_real = np.zeros((64, NK), np.float64)
        q_imag = np.zeros((64, NK), np.float64)
        y_real = np.zeros(64, np.float64)
        for c in range(N_CORES):
            out = results[c]
            q_real += out["o_qr"]
            q_imag += out["o_qi"]
            cs = out["o_cs"][0]
            cm = colmols[c]
            used = cm >= 0
            y_real += np.bincount(cm[used], weights=cs[used], minlength=64)

        recip = TWO_PI * np.transpose(invc, (0, 2, 1))     # [M,3,3]
        v_box = np.abs(np.linalg.det(cell.astype(np.float64)))
        prefactor = TWO_PI / v_box
        kv_m = np.einsum("kd,mde->mke", kv_use.astype(np.float64), recip)
        k_sq = np.sum(kv_m ** 2, axis=2)                   # [M,NK]
        q_gauss = np.exp(-0.25 * k_sq / ALPHA)
        q_dens = q_real ** 2 + q_imag ** 2
        y_ewald = prefactor * np.sum(wk * q_dens * q_gauss / k_sq, axis=1)
        self_int = math.sqrt(ALPHA / math.pi) * self_q2_host
        y = 0.5 * KE * y_real + KE * (y_ewald - self_int)
        return y.astype(np.float32)

    return nc, in_maps, combine


def kernel(**inputs):
    nc, in_maps, combine = prepare(inputs)
    res = bass_utils.run_bass_kernel_spmd(nc, in_maps,
                                          core_ids=list(range(N_CORES)))
    return combine(res.results)
